# Initial kernel scaffold
#
"""LorentzConv1d Trainium2 kernel (8-core data-parallel over batch).

Math (per batch element, L=8192, Cin=Cout=64, K=5, pad=2, k_curv=1):
  xp = pad(x, 2 each side along L); xp[...,0] clamped to >= 1
  t_resc[l] = sqrt(sum_{j=0..4} xp[l+j,0]^2 - 4)
  feats[l]  = [t_resc[l], xp[l+j, c] for c=1..63, j=0..4]   (316 features)
  y[l,o]    = feats[l] @ W[o,:] + b[o]
  out[l,0]  = sqrt(sum_{o>=1} y[l,o]^2 + 1);  out[l,o>=1] = y[l,o]

Kernel strategy (per core: 2 batch elements), v1:
  - x loaded fp32 per chunk (8 l-tiles) on the SP HWDGE ring, layout [p,t,c].
  - PE transposes x tiles (fp32) into PSUM; copy-out casts into S, a stacked
    bf16 im2col buffer [128, 8200]:
      row 0: t_resc[l], rows 1..63: space channels at shift 0,
      row 64: ones (bias, DMA'd from a DRAM const),
      rows 65..127: space channels at shift +1 (chunked SBUF->SBUF DMA;
      engines cannot cross partitions).
  - t_resc via banded-matrix matmuls on q=time^2 (fp32) in [128, 64] natural
    layout, sqrt on ACT, PE-transpose + DMA reshape into S row 0.
  - Conv: per 128-position tile, 3 bf16 matmuls with the S slice as the
    *stationary* operand (shifts +0,+2,+4) and [128, 63] weight packs as the
    moving operand. PSUM gets y[l, o] in natural [l, o] layout.
  - Epilogue: ACT copy PSUM->staging, GPSIMD square, DVE grouped reduce,
    ACT sqrt -> channel 0, fp32 store on the ACT HWDGE ring.
"""
import sys
import os

sys.path.insert(0, "/opt/trn_rl_repo")

import numpy as np
import ml_dtypes

BSZ, L, C = 16, 8192, 64
N_CORES = 8
B_PER_CORE = BSZ // N_CORES  # 2
KERNEL = 5
PAD = 2
K_CURV = 1.0
NT = L // 128          # 64 l-tiles per batch
CHUNK = 8              # l-tiles per chunk
NCK = NT // CHUNK      # 8 chunks per batch
SFREE = L + 8          # S free size (u=0..8195 used, +tail)

_cache = {}


def _build_consts(W, b):
    """numpy-side constant tensors."""
    bf16 = ml_dtypes.bfloat16
    W = np.asarray(W, np.float32)
    b = np.asarray(b, np.float32)
    # W[o, 1 + (c-1)*5 + j] -> Wr[o-1, c-1, j]
    Wr = W[1:, 1:].reshape(63, 63, KERNEL)
    Wp = np.zeros((3, 128, 63), np.float32)
    # MM1: shift 0 -> taps 0 (rows 1..63), 1 (rows 65..127); t_resc row 0; bias row 64
    Wp[0, 0, :] = W[1:, 0]
    Wp[0, 1:64, :] = Wr[:, :, 0].T
    Wp[0, 64, :] = b[1:]
    Wp[0, 65:128, :] = Wr[:, :, 1].T
    # MM2: shift +2 -> taps 2, 3
    Wp[1, 1:64, :] = Wr[:, :, 2].T
    Wp[1, 65:128, :] = Wr[:, :, 3].T
    # MM3: shift +4 -> tap 4
    Wp[2, 1:64, :] = Wr[:, :, 4].T

    k = np.arange(128)[:, None]
    m = np.arange(128)[None, :]
    band0 = ((k - m >= -2) & (k - m <= 2)).astype(np.float32)
    bandP = ((k - 128 - m >= -2) & (k - 128 - m <= -1)).astype(np.float32)
    bandN = ((k + 128 - m >= 1) & (k + 128 - m <= 2)).astype(np.float32)
    ident = np.eye(128, dtype=np.float32)
    ones_row = np.ones((1, SFREE), np.float32)
    return {
        "w_pack": Wp.astype(bf16),
        "band0": band0,
        "bandP": bandP,
        "bandN": bandN,
        "ident": ident,
        "ones_row": ones_row.astype(bf16),
    }


def _kernel_body(tc, out_ap, x_ap, consts):
    from contextlib import ExitStack
    import concourse.bass as bass
    from concourse import mybir

    bf16 = mybir.dt.bfloat16
    f32 = mybir.dt.float32
    AF = mybir.ActivationFunctionType
    nc = tc.nc

    with ExitStack() as ctx:
        singles = ctx.enter_context(tc.tile_pool(name="singles", bufs=1))
        xpool = ctx.enter_context(tc.tile_pool(name="xpool", bufs=2))
        spool = ctx.enter_context(tc.tile_pool(name="spool", bufs=2))
        qpool = ctx.enter_context(tc.tile_pool(name="qpool", bufs=2))
        opool = ctx.enter_context(tc.tile_pool(name="opool", bufs=3))
        zpool = ctx.enter_context(tc.tile_pool(name="zpool", bufs=2))
        pyp = ctx.enter_context(tc.tile_pool(name="pyp", bufs=3, space="PSUM"))
        ptp = ctx.enter_context(tc.tile_pool(name="ptp", bufs=3, space="PSUM"))
        psp = ctx.enter_context(tc.tile_pool(name="psp", bufs=1, space="PSUM"))

        # ---- load constants into SBUF
        wsb = singles.tile([128, 3, 63], bf16)
        nc.sync.dma_start(out=wsb, in_=consts["w_pack"].rearrange("t p o -> p t o"))
        band0 = singles.tile([128, 128], f32)
        nc.sync.dma_start(out=band0, in_=consts["band0"])
        bandP = singles.tile([128, 128], f32)
        nc.sync.dma_start(out=bandP, in_=consts["bandP"])
        bandN = singles.tile([128, 128], f32)
        nc.sync.dma_start(out=bandN, in_=consts["bandN"])
        ident = singles.tile([128, 128], f32)
        nc.sync.dma_start(out=ident, in_=consts["ident"])
        bias_m4 = singles.tile([128, 1], f32)
        nc.vector.memset(bias_m4, -(KERNEL - 1) * K_CURV)
        bias_p1 = singles.tile([128, 1], f32)
        nc.vector.memset(bias_p1, float(K_CURV))

        for b in range(B_PER_CORE):
            xr = x_ap[b].rearrange("(t p) c -> p t c", p=128)      # [128, 64, 64]
            orr = out_ap[b].rearrange("(t p) c -> p t c", p=128)   # [128, 64, 64]

            # ---- load fp32 per chunk (SP HWDGE ring)
            xb = xpool.tile([128, NT, C], f32)
            for ck in range(NCK):
                sl = slice(ck * CHUNK, (ck + 1) * CHUNK)
                nc.sync.dma_start(out=xb[:, sl, :], in_=xr[:, sl, :])

            # ---- S buffer: constant edges
            S = spool.tile([128, SFREE], bf16)
            nc.vector.memset(S[0:64, 0:2], 0.0)
            nc.vector.memset(S[0:64, L + 2:L + 5], 0.0)
            nc.vector.memset(S[0:1, L:L + 2], 0.0)
            nc.gpsimd.dma_start(out=S[64:65, :], in_=consts["ones_row"])

            # ---- q = time^2 (fp32) with boundary ones columns
            q_ext = qpool.tile([128, NT + 2], f32)
            nc.vector.memset(q_ext[:, 0:1], 1.0)
            nc.vector.memset(q_ext[:, NT + 1:NT + 2], 1.0)
            nc.vector.tensor_mul(q_ext[:, 1:NT + 1], xb[:, :, 0], xb[:, :, 0])

            # ---- t_resc = sqrt(window5(q) - 4) via banded matmuls
            ps_s = psp.tile([128, NT], f32)
            nc.tensor.matmul(ps_s, lhsT=band0, rhs=q_ext[:, 1:NT + 1],
                             start=True, stop=False)
            nc.tensor.matmul(ps_s, lhsT=bandP, rhs=q_ext[:, 0:NT],
                             start=False, stop=False)
            nc.tensor.matmul(ps_s, lhsT=bandN, rhs=q_ext[:, 2:NT + 2],
                             start=False, stop=True)
            t_resc = qpool.tile([128, NT], f32)
            nc.scalar.activation(t_resc, ps_s, AF.Sqrt, bias=bias_m4, scale=1.0)

            # transpose [128, 64] -> [64, 128] and flatten into S row 0
            psT2 = psp.tile([64, 128], f32)
            nc.tensor.transpose(psT2, t_resc, ident)
            trow = qpool.tile([64, 128], bf16)
            nc.scalar.copy(trow, psT2)

            # ---- transpose x tiles into S rows 0..63 (shift 0; row 0 = time,
            # overwritten by t_resc below). 4 tiles per PSUM bank (fp32).
            for ck in range(NCK):
                for h in range(2):
                    psT = ptp.tile([64, 4 * 128], f32)
                    for tt in range(4):
                        t = ck * CHUNK + h * 4 + tt
                        nc.tensor.transpose(psT[:, tt * 128:(tt + 1) * 128],
                                            xb[:, t, :], ident)
                    u0 = 2 + (ck * CHUNK + h * 4) * 128
                    dst = S[0:64, u0:u0 + 512]
                    if h == 0:
                        nc.scalar.copy(dst, psT)
                    else:
                        nc.vector.tensor_copy(dst, psT)

            # t_resc into S row 0 (overwrites the time row; taps for row 0 are 0)
            nc.gpsimd.dma_start(out=S[0:1, 0:L], in_=trow)

            # ---- shifted second copy (rows 65..127), chunked SBUF->SBUF DMA
            for ck in range(NCK):
                c0 = ck * 1024
                c1 = c0 + 1024 if ck < NCK - 1 else L + 4
                nc.gpsimd.dma_start(out=S[65:128, c0:c1], in_=S[1:64, c0 + 1:c1 + 1])

            # ---- conv + epilogue per chunk
            for ck in range(NCK):
                py = pyp.tile([128, CHUNK, 63], f32)
                for tt in range(CHUNK):
                    t = ck * CHUNK + tt
                    u0 = t * 128
                    o = py[:, tt, :]
                    nc.tensor.matmul(o, lhsT=S[:, u0:u0 + 128],
                                     rhs=wsb[:, 0, :], start=True, stop=False)
                    nc.tensor.matmul(o, lhsT=S[:, u0 + 2:u0 + 130],
                                     rhs=wsb[:, 1, :], start=False, stop=False)
                    nc.tensor.matmul(o, lhsT=S[:, u0 + 4:u0 + 132],
                                     rhs=wsb[:, 2, :], start=False, stop=True)

                stag = opool.tile([128, CHUNK, 64], f32)
                nc.scalar.activation(stag[:, :, 1:64], py, AF.Copy)
                z = zpool.tile([128, CHUNK, 63], f32)
                nc.gpsimd.tensor_mul(z, stag[:, :, 1:64], stag[:, :, 1:64])
                yt2 = zpool.tile([128, CHUNK], f32)
                nc.vector.tensor_reduce(yt2, z, axis=mybir.AxisListType.X,
                                        op=mybir.AluOpType.add)
                nc.scalar.activation(stag[:, :, 0], yt2, AF.Sqrt,
                                     bias=bias_p1, scale=1.0)
                # store on the ACT HWDGE ring (decoupled from loads on SP)
                nc.scalar.dma_start(out=orr[:, ck * CHUNK:(ck + 1) * CHUNK, :],
                                    in_=stag)


def _build():
    if "nc" in _cache:
        return _cache["nc"]
    import concourse.bacc as bacc
    import concourse.tile as tile
    from concourse import mybir

    bf16 = mybir.dt.bfloat16
    f32 = mybir.dt.float32
    nc = bacc.Bacc("TRN2", target_bir_lowering=False, debug=False,
                   num_devices=N_CORES)
    x_in = nc.dram_tensor("x_shard", (B_PER_CORE, L, C), f32,
                          kind="ExternalInput").ap()
    w_pack = nc.dram_tensor("w_pack", (3, 128, 63), bf16,
                            kind="ExternalInput").ap()
    band0 = nc.dram_tensor("band0", (128, 128), f32, kind="ExternalInput").ap()
    bandP = nc.dram_tensor("bandP", (128, 128), f32, kind="ExternalInput").ap()
    bandN = nc.dram_tensor("bandN", (128, 128), f32, kind="ExternalInput").ap()
    ident = nc.dram_tensor("ident", (128, 128), f32, kind="ExternalInput").ap()
    ones_row = nc.dram_tensor("ones_row", (1, SFREE), bf16,
                              kind="ExternalInput").ap()
    out = nc.dram_tensor("out_shard", (B_PER_CORE, L, C), f32,
                         kind="ExternalOutput").ap()
    consts = {"w_pack": w_pack, "band0": band0, "bandP": bandP,
              "bandN": bandN, "ident": ident, "ones_row": ones_row}
    with tile.TileContext(nc) as tc:
        _kernel_body(tc, out, x_in, consts)
    nc.compile()
    _cache["nc"] = nc
    return nc


def _run(x, W, b, trace=False):
    from concourse.bass_utils import run_bass_kernel_spmd

    nc = _build()
    x = np.ascontiguousarray(np.asarray(x, np.float32))
    consts = _build_consts(W, b)
    in_maps = []
    for c in range(N_CORES):
        m = {"x_shard": np.ascontiguousarray(x[c * B_PER_CORE:(c + 1) * B_PER_CORE])}
        m.update(consts)
        in_maps.append(m)
    res = run_bass_kernel_spmd(nc, in_maps, list(range(N_CORES)), trace=trace)
    out = np.concatenate([res.results[c]["out_shard"] for c in range(N_CORES)],
                         axis=0)
    return out, res


def kernel(x, W, b):
    out, _ = _run(x, W, b, trace=False)
    return out


def kernel_timed(x, W, b):
    out, res = _run(x, W, b, trace=True)
    return out, res



# revision 15
# speedup vs baseline: 1.1983x; 1.1983x over previous
"""LorentzConv1d Trainium2 kernel (8-core data-parallel over batch), v2.

Math (per batch element, L=8192, Cin=Cout=64, K=5, pad=2, k_curv=1):
  xp = pad(x, 2 each side along L); xp[...,0] clamped to >= 1
  t_resc[l] = sqrt(sum_{j=0..4} xp[l+j,0]^2 - 4)
  feats[l]  = [t_resc[l], xp[l-2+j, c] for c=1..63, j=0..4]   (316 features)
  y[l,o]    = feats[l] @ W[o,:] + b[o]
  out[l,0]  = sqrt(sum_{o>=1} y[l,o]^2 + 1);  out[l,o>=1] = y[l,o]

v2 strategy (per core: 2 batch elements):
  - x loaded BLOCKED (partition p = l//64, free (t=l%64, c)) via gpsimd
    SWDGE with fp32->bf16 cast in flight: 4KB contiguous DRAM descriptors.
  - q = time^2 in blocked layout; t_resc window-sum via shifted adds along
    the free dim with a 2-column cross-partition halo DMA; ACT sqrt(-4);
    DMA into S row 0 (128B descs, per-partition contiguous).
  - S is a 65-row im2col buffer [65, 8200] bf16: row 0 t_resc (col=l),
    rows 1..63 space channels (col=l+2), row 64 ones (bias).
    Assembly: 64 bf16 PE transposes of [128,64] blocked slices into PSUM
    (4 per bank), then strided engine copies (alternating ACT/DVE) into S.
  - Conv: per output slot, 5 bf16 matmuls with 65-row contraction
    (one per tap); lhsT = S columns at stride 2 (pair-permutation sigma:
    PSUM partition p holds position base+2p+j), rhs = [65,64] weight packs
    (out col 0 zeroed).
  - Pair-permutation makes the fp32 store use 512B DRAM descriptors
    (two consecutive rows per partition) - 2x fewer DMA descriptor-ns.
  - Epilogue per double-chunk (2048 positions): ACT copy PSUM->stag,
    square (ACT/DVE alternating), DVE grouped reduce, ACT sqrt -> stag
    col 0, store on the SP HWDGE ring.
"""
import sys
import os

sys.path.insert(0, "/opt/trn_rl_repo")

import numpy as np
import ml_dtypes

BSZ, L, C = 16, 8192, 64
N_CORES = 8
B_PER_CORE = BSZ // N_CORES  # 2
KERNEL = 5
PAD = 2
K_CURV = 1.0
NT = 64                # t index within a partition block (l = 64*p + t)
NBLK = 68              # S2 column blocks: blk = t + 2, two halo blocks per side
SFREE = NBLK * 128 + 256   # 8960: +256 tail so conv rearrange views stay in-bounds
NDC = 4                # double-chunks per batch (2048 positions each)

_cache = {}


def _build_consts(W, b):
    """numpy-side constant tensors."""
    bf16 = ml_dtypes.bfloat16
    W = np.asarray(W, np.float32)
    b = np.asarray(b, np.float32)
    # W[o, 1 + (c-1)*5 + j] -> Wr[o-1, c-1, j]
    Wr = W[1:, 1:].reshape(63, 63, KERNEL)
    # 5 packs of [65, 64]: pack s = tap s; rows: 0 t_resc (s=0 only),
    # 1..63 space channels, 64 bias (s=0 only). Out col 0 is zero.
    Wp = np.zeros((KERNEL, 65, 64), np.float32)
    for s in range(KERNEL):
        Wp[s, 1:64, 1:] = Wr[:, :, s].T
    Wp[0, 0, 1:] = W[1:, 0]
    Wp[0, 64, 1:] = b[1:]

    ident = np.eye(128, dtype=np.float32)
    ones_row = np.ones((1, SFREE), np.float32)
    return {
        "w_pack": Wp.astype(bf16),
        "ident_bf": ident.astype(bf16),
        "ones_row": ones_row.astype(bf16),
    }


def _kernel_body(tc, out_ap, x_ap, consts):
    from contextlib import ExitStack
    import concourse.bass as bass
    from concourse import mybir

    bf16 = mybir.dt.bfloat16
    f32 = mybir.dt.float32
    AF = mybir.ActivationFunctionType
    nc = tc.nc

    with ExitStack() as ctx:
        singles = ctx.enter_context(tc.tile_pool(name="singles", bufs=1))
        xpool = ctx.enter_context(tc.tile_pool(name="xpool", bufs=2))
        spool = ctx.enter_context(tc.tile_pool(name="spool", bufs=2))
        qpool = ctx.enter_context(tc.tile_pool(name="qpool", bufs=2))
        zpool = ctx.enter_context(tc.tile_pool(name="zpool", bufs=2))
        stpool = ctx.enter_context(tc.tile_pool(name="stpool", bufs=3))
        ptp = ctx.enter_context(tc.tile_pool(name="ptp", bufs=3, space="PSUM"))
        pttp = ctx.enter_context(tc.tile_pool(name="pttp", bufs=1, space="PSUM"))
        pyp = ctx.enter_context(tc.tile_pool(name="pyp", bufs=2, space="PSUM"))

        # ---- load constants into SBUF
        wsb = singles.tile([65, KERNEL, 64], bf16)
        nc.sync.dma_start(out=wsb, in_=consts["w_pack"].rearrange("s p o -> p s o"))
        ident = singles.tile([128, 128], bf16)
        nc.sync.dma_start(out=ident, in_=consts["ident_bf"])
        bias_m4 = singles.tile([128, 1], f32)
        nc.vector.memset(bias_m4, -(KERNEL - 1) * K_CURV)
        bias_p1 = singles.tile([128, 1], f32)
        nc.vector.memset(bias_p1, float(K_CURV))
        # First ACT op is a Sqrt so the loaded function set (sqrt_and_others)
        # covers Copy/Square/Sqrt - avoids a mid-kernel ACT_TABLE_LOAD.
        nc.scalar.activation(bias_p1, bias_p1, AF.Sqrt)

        # Phase 1: S assembly for BOTH batches (loads, transposes, copies,
        # t_resc).  Phase 2: conv+epilogue for both.  Hoisting batch 1's
        # assembly ahead of batch 0's conv keeps the PE queue dense: the
        # conv phases then run back-to-back, letting the PE p-state ramp.
        Ss = [None] * B_PER_CORE
        for b in range(B_PER_CORE):
            # blocked layout: row l = 64*p + t
            xr = x_ap[b].rearrange("(p t) c -> p t c", p=128)      # [128,64,64]

            # ---- S2 im2col buffer, block-major: col = blk*128 + P holds
            #      x[64P + blk - 2] (blk=t+2; blk 0,1 / 66,67 are halos).
            #      Row 0: t_resc[64P + blk] for blk<64. Row 64: ones.
            S = spool.tile([65, SFREE], bf16)
            # pad columns: (blk 0,1, P=0) = x[-2,-1]; (blk 66,67, P=127) = x[8192,8193]
            nc.gpsimd.memset(
                S[0:64, 0:256].rearrange("c (b p) -> c b p", b=2)[:, :, 0:1], 0.0)
            nc.gpsimd.memset(
                S[0:64, 66 * 128:68 * 128].rearrange(
                    "c (b p) -> c b p", b=2)[:, :, 127:128], 0.0)
            nc.sync.dma_start(out=S[64:65, :], in_=consts["ones_row"])

            # ---- blocked bf16 x load (gpsimd SWDGE casts f32->bf16),
            #      interleaved with transposes + contiguous copies into S
            xb = xpool.tile([128, NT, C], bf16)
            for lk in range(4):
                sl = slice(lk * 16, (lk + 1) * 16)
                nc.gpsimd.dma_start(out=xb[:, sl, :], in_=xr[:, sl, :])
                for g in range(4 * lk, 4 * lk + 4):
                    psT = ptp.tile([64, 4, 128], bf16)
                    for tt in range(4):
                        t = 4 * g + tt
                        nc.tensor.transpose(psT[:, tt, :], xb[:, t, :], ident)
                    # contiguous [64, 512] copy: blocks 4g+2 .. 4g+5
                    dst = S[0:64, (4 * g + 2) * 128:(4 * g + 6) * 128]
                    dst = dst.rearrange("c (b p) -> c b p", b=4)
                    if g % 2 == 0:
                        nc.scalar.copy(dst, psT)
                    else:
                        nc.vector.tensor_copy(dst, psT)
                    if g == 0:
                        # high halo: blocks 66,67 (P 0..126) <- t=0,1 @ p 1..127
                        dh = S[0:64, 66 * 128:68 * 128].rearrange(
                            "c (b p) -> c b p", b=2)[:, :, 0:127]
                        nc.vector.tensor_copy(dh, psT[:, 0:2, 1:128])
                    if g == 15:
                        # low halo: blocks 0,1 (P 1..127) <- t=62,63 @ p 0..126
                        dl = S[0:64, 0:256].rearrange(
                            "c (b p) -> c b p", b=2)[:, :, 1:128]
                        nc.scalar.copy(dl, psT[:, 2:4, 0:127])

            # ---- t_resc path: q = time^2 with halo, window-sum, sqrt(-4)
            q = qpool.tile([128, NT + 4], f32)
            nc.vector.memset(q[:, 0:2], 1.0)
            nc.vector.memset(q[:, 66:68], 1.0)
            nc.vector.tensor_mul(q[:, 2:66], xb[:, :, 0], xb[:, :, 0])
            nc.sync.dma_start(out=q[1:128, 0:2], in_=q[0:127, 64:66])
            nc.sync.dma_start(out=q[0:127, 66:68], in_=q[1:128, 2:4])
            tsum = qpool.tile([128, NT], f32)
            nc.vector.tensor_add(tsum, q[:, 0:64], q[:, 1:65])
            nc.vector.tensor_add(tsum, tsum, q[:, 2:66])
            nc.vector.tensor_add(tsum, tsum, q[:, 3:67])
            nc.vector.tensor_add(tsum, tsum, q[:, 4:68])
            t_resc = qpool.tile([128, NT], bf16)
            nc.scalar.activation(t_resc, tsum, AF.Sqrt, bias=bias_m4, scale=1.0)
            # S row 0 is block-major: transpose t_resc to [t, p] first, then
            # one contiguous DMA (64 x 256B descs) into cols 0..8191.
            psTt = pttp.tile([64, 128], bf16)
            nc.tensor.transpose(psTt, t_resc, ident)
            tTs = qpool.tile([64, 128], bf16)
            nc.scalar.copy(tTs, psTt)
            nc.sync.dma_start(out=S[0:1, 0:L], in_=tTs)
            Ss[b] = S

        for b in range(B_PER_CORE):
            S = Ss[b]
            # store: row l = 64*p + 16*dk + t  (fully contiguous per
            # partition: 4KB descriptors per double-chunk store)
            orr = out_ap[b].rearrange("(p dk t) c -> p dk (t c)",
                                      p=128, dk=NDC)               # [128,4,1024]
            # ---- conv + epilogue per double-chunk (2048 positions)
            for dk in range(NDC):
                py = pyp.tile([128, 16, 64], f32)
                for tt in range(16):
                    t0 = 16 * dk + tt
                    o = py[:, tt, :]
                    # out partition P <-> position l = 64P + t0; tap s
                    # reads the contiguous column block blk = t0 + s.
                    for s in range(KERNEL):
                        lhsT = S[0:65, (t0 + s) * 128:(t0 + s + 1) * 128]
                        nc.tensor.matmul(o, lhsT=lhsT, rhs=wsb[:, s, :],
                                         start=(s == 0),
                                         stop=(s == KERNEL - 1))

                stag = stpool.tile([128, 16, 64], f32)
                nc.scalar.copy(stag, py)
                z = zpool.tile([128, 16, 63], f32)
                if dk % 2 == 0:
                    nc.scalar.activation(z, stag[:, :, 1:64], AF.Square)
                else:
                    nc.vector.tensor_mul(z, stag[:, :, 1:64], stag[:, :, 1:64])
                yt2 = zpool.tile([128, 16], f32)
                nc.vector.tensor_reduce(yt2, z, axis=mybir.AxisListType.X,
                                        op=mybir.AluOpType.add)
                nc.scalar.activation(stag[:, :, 0], yt2, AF.Sqrt,
                                     bias=bias_p1, scale=1.0)
                nc.sync.dma_start(out=orr[:, dk, :],
                                  in_=stag.rearrange("p t c -> p (t c)"))


def _build():
    if "nc" in _cache:
        return _cache["nc"]
    import concourse.bacc as bacc
    import concourse.tile as tile
    from concourse import mybir

    bf16 = mybir.dt.bfloat16
    f32 = mybir.dt.float32
    nc = bacc.Bacc("TRN2", target_bir_lowering=False, debug=False,
                   num_devices=N_CORES)
    x_in = nc.dram_tensor("x_shard", (B_PER_CORE, L, C), f32,
                          kind="ExternalInput").ap()
    w_pack = nc.dram_tensor("w_pack", (KERNEL, 65, 64), bf16,
                            kind="ExternalInput").ap()
    ident_bf = nc.dram_tensor("ident_bf", (128, 128), bf16,
                              kind="ExternalInput").ap()
    ones_row = nc.dram_tensor("ones_row", (1, SFREE), bf16,
                              kind="ExternalInput").ap()
    out = nc.dram_tensor("out_shard", (B_PER_CORE, L, C), f32,
                         kind="ExternalOutput").ap()
    consts = {"w_pack": w_pack, "ident_bf": ident_bf, "ones_row": ones_row}
    with tile.TileContext(nc) as tc:
        _kernel_body(tc, out, x_in, consts)
    nc.compile()
    _cache["nc"] = nc
    return nc


def _run(x, W, b, trace=False):
    from concourse.bass_utils import run_bass_kernel_spmd

    nc = _build()
    x = np.ascontiguousarray(np.asarray(x, np.float32))
    consts = _build_consts(W, b)
    in_maps = []
    for c in range(N_CORES):
        m = {"x_shard": np.ascontiguousarray(x[c * B_PER_CORE:(c + 1) * B_PER_CORE])}
        m.update(consts)
        in_maps.append(m)
    res = run_bass_kernel_spmd(nc, in_maps, list(range(N_CORES)), trace=trace)
    out = np.concatenate([res.results[c]["out_shard"] for c in range(N_CORES)],
                         axis=0)
    return out, res


def kernel(x, W, b):
    out, _ = _run(x, W, b, trace=False)
    return out


def kernel_timed(x, W, b):
    out, res = _run(x, W, b, trace=True)
    return out, res


# revision 18
# speedup vs baseline: 1.2757x; 1.0646x over previous
"""LorentzConv1d Trainium2 kernel (8-core data-parallel over batch), v2.

Math (per batch element, L=8192, Cin=Cout=64, K=5, pad=2, k_curv=1):
  xp = pad(x, 2 each side along L); xp[...,0] clamped to >= 1
  t_resc[l] = sqrt(sum_{j=0..4} xp[l+j,0]^2 - 4)
  feats[l]  = [t_resc[l], xp[l-2+j, c] for c=1..63, j=0..4]   (316 features)
  y[l,o]    = feats[l] @ W[o,:] + b[o]
  out[l,0]  = sqrt(sum_{o>=1} y[l,o]^2 + 1);  out[l,o>=1] = y[l,o]

v2 strategy (per core: 2 batch elements):
  - x loaded BLOCKED (partition p = l//64, free (t=l%64, c)) via gpsimd
    SWDGE with fp32->bf16 cast in flight: 4KB contiguous DRAM descriptors.
  - q = time^2 in blocked layout; t_resc window-sum via shifted adds along
    the free dim with a 2-column cross-partition halo DMA; ACT sqrt(-4);
    DMA into S row 0 (128B descs, per-partition contiguous).
  - S is a 65-row im2col buffer [65, 8200] bf16: row 0 t_resc (col=l),
    rows 1..63 space channels (col=l+2), row 64 ones (bias).
    Assembly: 64 bf16 PE transposes of [128,64] blocked slices into PSUM
    (4 per bank), then strided engine copies (alternating ACT/DVE) into S.
  - Conv: per output slot, 5 bf16 matmuls with 65-row contraction
    (one per tap); lhsT = S columns at stride 2 (pair-permutation sigma:
    PSUM partition p holds position base+2p+j), rhs = [65,64] weight packs
    (out col 0 zeroed).
  - Pair-permutation makes the fp32 store use 512B DRAM descriptors
    (two consecutive rows per partition) - 2x fewer DMA descriptor-ns.
  - Epilogue per double-chunk (2048 positions): ACT copy PSUM->stag,
    square (ACT/DVE alternating), DVE grouped reduce, ACT sqrt -> stag
    col 0, store on the SP HWDGE ring.
"""
import sys
import os

sys.path.insert(0, "/opt/trn_rl_repo")

import numpy as np
import ml_dtypes

BSZ, L, C = 16, 8192, 64
N_CORES = 8
B_PER_CORE = BSZ // N_CORES  # 2
KERNEL = 5
PAD = 2
K_CURV = 1.0
NT = 64                # t index within a partition block (l = 64*p + t)
NBLK = 68              # S2 column blocks: blk = t + 2, two halo blocks per side
SFREE = NBLK * 128 + 256   # 8960: +256 tail so conv rearrange views stay in-bounds
NCH = 8                # conv chunks per batch (8 slots = 512 positions)

_cache = {}


def _build_consts(W, b):
    """numpy-side constant tensors."""
    bf16 = ml_dtypes.bfloat16
    W = np.asarray(W, np.float32)
    b = np.asarray(b, np.float32)
    # W[o, 1 + (c-1)*5 + j] -> Wr[o-1, c-1, j]
    Wr = W[1:, 1:].reshape(63, 63, KERNEL)
    # 5 packs of [65, 64] stored tap-REVERSED (pack m = tap 4-m):
    # rows: 0 t_resc (tap 0 only), 1..63 space channels, 64 bias (tap 0
    # only). Out col 0 is zero. Reversed order makes the per-block
    # multi-slot accumulate matmul's rhs a contiguous slice.
    Wp = np.zeros((KERNEL, 65, 64), np.float32)
    for s in range(KERNEL):
        m = KERNEL - 1 - s
        Wp[m, 1:64, 1:] = Wr[:, :, s].T
    Wp[KERNEL - 1, 0, 1:] = W[1:, 0]
    Wp[KERNEL - 1, 64, 1:] = b[1:]

    ident = np.eye(128, dtype=np.float32)
    ones_row = np.ones((1, SFREE), np.float32)
    return {
        "w_pack": Wp.astype(bf16),
        "ident_bf": ident.astype(bf16),
        "ones_row": ones_row.astype(bf16),
    }


def _kernel_body(tc, out_ap, x_ap, consts):
    from contextlib import ExitStack
    import concourse.bass as bass
    from concourse import mybir

    bf16 = mybir.dt.bfloat16
    f32 = mybir.dt.float32
    AF = mybir.ActivationFunctionType
    nc = tc.nc

    with ExitStack() as ctx:
        singles = ctx.enter_context(tc.tile_pool(name="singles", bufs=1))
        xpool = ctx.enter_context(tc.tile_pool(name="xpool", bufs=2))
        spool = ctx.enter_context(tc.tile_pool(name="spool", bufs=2))
        qpool = ctx.enter_context(tc.tile_pool(name="qpool", bufs=2))
        zpool = ctx.enter_context(tc.tile_pool(name="zpool", bufs=2))
        stpool = ctx.enter_context(tc.tile_pool(name="stpool", bufs=3))
        ptp = ctx.enter_context(tc.tile_pool(name="ptp", bufs=4, space="PSUM"))
        pttp = ctx.enter_context(tc.tile_pool(name="pttp", bufs=1, space="PSUM"))
        pyp = ctx.enter_context(tc.tile_pool(name="pyp", bufs=3, space="PSUM"))

        # ---- load constants into SBUF
        wsb = singles.tile([65, KERNEL, 64], bf16)
        nc.sync.dma_start(out=wsb, in_=consts["w_pack"].rearrange("s p o -> p s o"))
        ident = singles.tile([128, 128], bf16)
        nc.sync.dma_start(out=ident, in_=consts["ident_bf"])
        bias_m4 = singles.tile([128, 1], f32)
        nc.vector.memset(bias_m4, -(KERNEL - 1) * K_CURV)
        bias_p1 = singles.tile([128, 1], f32)
        nc.vector.memset(bias_p1, float(K_CURV))
        # First ACT op is a Sqrt so the loaded function set (sqrt_and_others)
        # covers Copy/Square/Sqrt - avoids a mid-kernel ACT_TABLE_LOAD.
        nc.scalar.activation(bias_p1, bias_p1, AF.Sqrt)

        # Phase 1: S assembly for BOTH batches (loads, transposes, copies,
        # t_resc).  Phase 2: conv+epilogue for both.  Hoisting batch 1's
        # assembly ahead of batch 0's conv keeps the PE queue dense: the
        # conv phases then run back-to-back, letting the PE p-state ramp.
        Ss = [None] * B_PER_CORE
        for b in range(B_PER_CORE):
            # blocked layout: row l = 64*p + t
            xr = x_ap[b].rearrange("(p t) c -> p t c", p=128)      # [128,64,64]

            # ---- S2 im2col buffer, block-major: col = blk*128 + P holds
            #      x[64P + blk - 2] (blk=t+2; blk 0,1 / 66,67 are halos).
            #      Row 0: t_resc[64P + blk] for blk<64. Row 64: ones.
            S = spool.tile([65, SFREE], bf16)
            # pad columns: (blk 0,1, P=0) = x[-2,-1]; (blk 66,67, P=127) = x[8192,8193]
            nc.gpsimd.memset(
                S[0:64, 0:256].rearrange("c (b p) -> c b p", b=2)[:, :, 0:1], 0.0)
            nc.gpsimd.memset(
                S[0:64, 66 * 128:68 * 128].rearrange(
                    "c (b p) -> c b p", b=2)[:, :, 127:128], 0.0)
            nc.sync.dma_start(out=S[64:65, :], in_=consts["ones_row"])

            # ---- blocked bf16 x load (gpsimd SWDGE casts f32->bf16),
            #      interleaved with transposes + contiguous copies into S
            xb = xpool.tile([128, NT, C], bf16)
            for lk in range(8):
                sl = slice(lk * 8, (lk + 1) * 8)
                nc.gpsimd.dma_start(out=xb[:, sl, :], in_=xr[:, sl, :])
                for g in range(2 * lk, 2 * lk + 2):
                    psT = ptp.tile([64, 4, 128], bf16)
                    for tt in range(4):
                        t = 4 * g + tt
                        nc.tensor.transpose(psT[:, tt, :], xb[:, t, :], ident)
                    # contiguous [64, 512] copy: blocks 4g+2 .. 4g+5
                    dst = S[0:64, (4 * g + 2) * 128:(4 * g + 6) * 128]
                    dst = dst.rearrange("c (b p) -> c b p", b=4)
                    if g % 2 == 0:
                        nc.scalar.copy(dst, psT)
                    else:
                        nc.vector.tensor_copy(dst, psT)
                    if g == 0:
                        # high halo: blocks 66,67 (P 0..126) <- t=0,1 @ p 1..127
                        dh = S[0:64, 66 * 128:68 * 128].rearrange(
                            "c (b p) -> c b p", b=2)[:, :, 0:127]
                        nc.vector.tensor_copy(dh, psT[:, 0:2, 1:128])
                    if g == 15:
                        # low halo: blocks 0,1 (P 1..127) <- t=62,63 @ p 0..126
                        dl = S[0:64, 0:256].rearrange(
                            "c (b p) -> c b p", b=2)[:, :, 1:128]
                        nc.scalar.copy(dl, psT[:, 2:4, 0:127])

            # ---- t_resc path: q = time^2 with halo, window-sum, sqrt(-4)
            q = qpool.tile([128, NT + 4], f32)
            nc.vector.memset(q[:, 0:2], 1.0)
            nc.vector.memset(q[:, 66:68], 1.0)
            nc.vector.tensor_mul(q[:, 2:66], xb[:, :, 0], xb[:, :, 0])
            nc.sync.dma_start(out=q[1:128, 0:2], in_=q[0:127, 64:66])
            nc.sync.dma_start(out=q[0:127, 66:68], in_=q[1:128, 2:4])
            tsum = qpool.tile([128, NT], f32)
            nc.vector.tensor_add(tsum, q[:, 0:64], q[:, 1:65])
            nc.vector.tensor_add(tsum, tsum, q[:, 2:66])
            nc.vector.tensor_add(tsum, tsum, q[:, 3:67])
            nc.vector.tensor_add(tsum, tsum, q[:, 4:68])
            t_resc = qpool.tile([128, NT], bf16)
            nc.scalar.activation(t_resc, tsum, AF.Sqrt, bias=bias_m4, scale=1.0)
            # S row 0 is block-major: transpose t_resc to [t, p] first, then
            # one contiguous DMA (64 x 256B descs) into cols 0..8191.
            psTt = pttp.tile([64, 128], bf16)
            nc.tensor.transpose(psTt, t_resc, ident)
            tTs = qpool.tile([64, 128], bf16)
            nc.scalar.copy(tTs, psTt)
            nc.sync.dma_start(out=S[0:1, 0:L], in_=tTs)
            Ss[b] = S

        for b in range(B_PER_CORE):
            S = Ss[b]
            # store: row l = 64*p + 16*dk + t  (fully contiguous per
            # partition: 4KB descriptors per double-chunk store)
            orr = out_ap[b].rearrange("(p dk t) c -> p dk (t c)",
                                      p=128, dk=NCH)               # [128,8,512]
            # ---- conv + epilogue per chunk (8 slots, one PSUM bank)
            for dk in range(NCH):
                py = pyp.tile([128, 8, 64], f32)
                # Block-major accumulation: for each S column block blk,
                # load the stationary once-ish: a "start" matmul writes tap 0
                # of slot blk, then one wide matmul accumulates taps 1..4
                # into slots blk-4..blk-1 (2 LD_WEIGHTS per block vs 5).
                # Slot t0 (position l = 64P + t0) receives tap s from block
                # t0+s; rhs pack m = 4-s, so slots t_lo..t_hi of block blk
                # use the contiguous rhs slice m = t_lo-blk+4 .. t_hi-blk+4.
                for tt in range(8):
                    t0 = 8 * dk + tt
                    o = py[:, tt, :]
                    # out partition P <-> position l = 64P + t0; tap s reads
                    # contiguous block t0+s; packs are tap-reversed (m=4-s).
                    for s in range(KERNEL):
                        lhsT = S[0:65, (t0 + s) * 128:(t0 + s + 1) * 128]
                        nc.tensor.matmul(o, lhsT=lhsT,
                                         rhs=wsb[:, KERNEL - 1 - s, :],
                                         start=(s == 0),
                                         stop=(s == KERNEL - 1))

                stag = stpool.tile([128, 8, 64], f32)
                nc.scalar.copy(stag, py)
                z = zpool.tile([128, 8, 63], f32)
                if dk % 2 == 0:
                    nc.scalar.activation(z, stag[:, :, 1:64], AF.Square)
                else:
                    nc.vector.tensor_mul(z, stag[:, :, 1:64], stag[:, :, 1:64])
                yt2 = zpool.tile([128, 8], f32)
                nc.vector.tensor_reduce(yt2, z, axis=mybir.AxisListType.X,
                                        op=mybir.AluOpType.add)
                nc.scalar.activation(stag[:, :, 0], yt2, AF.Sqrt,
                                     bias=bias_p1, scale=1.0)
                nc.sync.dma_start(out=orr[:, dk, :],
                                  in_=stag.rearrange("p t c -> p (t c)"))


def _build():
    if "nc" in _cache:
        return _cache["nc"]
    import concourse.bacc as bacc
    import concourse.tile as tile
    from concourse import mybir

    bf16 = mybir.dt.bfloat16
    f32 = mybir.dt.float32
    nc = bacc.Bacc("TRN2", target_bir_lowering=False, debug=False,
                   num_devices=N_CORES)
    x_in = nc.dram_tensor("x_shard", (B_PER_CORE, L, C), f32,
                          kind="ExternalInput").ap()
    w_pack = nc.dram_tensor("w_pack", (KERNEL, 65, 64), bf16,
                            kind="ExternalInput").ap()
    ident_bf = nc.dram_tensor("ident_bf", (128, 128), bf16,
                              kind="ExternalInput").ap()
    ones_row = nc.dram_tensor("ones_row", (1, SFREE), bf16,
                              kind="ExternalInput").ap()
    out = nc.dram_tensor("out_shard", (B_PER_CORE, L, C), f32,
                         kind="ExternalOutput").ap()
    consts = {"w_pack": w_pack, "ident_bf": ident_bf, "ones_row": ones_row}
    with tile.TileContext(nc) as tc:
        _kernel_body(tc, out, x_in, consts)
    nc.compile()
    _cache["nc"] = nc
    return nc


def _run(x, W, b, trace=False):
    from concourse.bass_utils import run_bass_kernel_spmd

    nc = _build()
    x = np.ascontiguousarray(np.asarray(x, np.float32))
    consts = _build_consts(W, b)
    in_maps = []
    for c in range(N_CORES):
        m = {"x_shard": np.ascontiguousarray(x[c * B_PER_CORE:(c + 1) * B_PER_CORE])}
        m.update(consts)
        in_maps.append(m)
    res = run_bass_kernel_spmd(nc, in_maps, list(range(N_CORES)), trace=trace)
    out = np.concatenate([res.results[c]["out_shard"] for c in range(N_CORES)],
                         axis=0)
    return out, res


def kernel(x, W, b):
    out, _ = _run(x, W, b, trace=False)
    return out


def kernel_timed(x, W, b):
    out, res = _run(x, W, b, trace=True)
    return out, res
